# revision 68
# baseline (speedup 1.0000x reference)
"""Trainium2 Bass kernel: 16-head MHA (B=2, T=2048, D=1024, d_k=64).

Sharding (8 NeuronCores): data-parallel over the batch (2) x tensor-parallel
over head groups (4 groups of 4 heads).  Core c handles batch b = c//4 and
heads [4g, 4g+4) with g = c%4.  Each core computes its partial output
    sum_{h in group} softmax((q Wq_h + bq_h)(k Wk_h)^T / 8) (v Wv_h) Wo_h
and the host sums the 4 partials per batch and adds the constant row
bo + bv @ Wo once.  bk is dropped: with the all-ones mask it shifts every
score row by a per-row constant, which softmax ignores exactly.

Design notes (420us baseline -> ~211us):
  * every matmul operand is bf16 (FWL weight loads, fp32 PSUM
    accumulate); output DMA'd as bf16 and upconverted host-side.
    The two heads of a pair sit on partition halves 0:64 / 64:128, so
    their C=64 scores matmuls land on disjoint PE row-tiles (T0/T8) and
    execute CONCURRENTLY (measured: starts 3 ns apart) -- scores cost
    half the naive streaming time.  (Column-tiled pair splits of the
    C=128 projections were tried and serialize; only row tiles overlap.)
  * V is projected directly in [t, v-col] layout (stationary = x^T
    chunk, moving = Wv) -- no PE transposes.  Activations arrive via
    host-packed chunk-major layouts (xq/xk per 512-col group, xv per
    128-row k-tile) so each projection unit depends on ~1 MB of DMA,
    not the whole tensor.
  * attention runs in 512-wide q stripes; per (stripe, head-pair, kt):
    2 concurrent scores MMs -> one [128,1024] fp32 PSUM tile, one ACT
    exp -> bf16 es, 2 aV MMs accumulating into per-head [65,512] PSUM
    (the 65th V_ext ones-column yields softmax rowsums for free, which
    is column-optimal: a separate rowsum matmul would stream es again).
    PSUM: scores dbuf 2x2 banks + O-accum 2 + filler 2 = 8 exactly.
  * one flat software-pipelined (stripe, hp, kt) stream: aV runs 1-3
    steps behind scores/exp (deepened at block starts) so the FIFO PE
    queue never waits on an exp or on the previous block's drains.
  * the PE is kept dense -- and the HAM clock gate at 2.4 GHz -- by
    emitting everything else as fillers inside the ACT-bound attention
    loop: K groups 1-3 + V tiles 6-15 + Q stripe s+1 projections, the
    reciprocal dance, and stripe s-1's output projection, each unit
    ordered so its drain is emitted before its first consumer.
    ~150 dependency-free warm-up matmuls cover the ~7us DMA startup.
  * softmax denominators: rowsum rows are collected at partitions 32h,
    DVE 32-block transposed, one batched ACT Ln + Exp(-1) computes all
    reciprocals partition-parallel, transposed back, and broadcast
    across partitions by one C=128 selector matmul per head pair.
    The last stripe runs hp0's dance under hp1's attention and drains
    on the otherwise-idle ACT queue to shorten the serial tail.
"""

import functools
import os
from collections import deque

import ml_dtypes
import numpy as np

import concourse.bass as bass
import concourse.mybir as mybir
import concourse.tile as tile
from concourse import bacc
from concourse.bass_utils import run_bass_kernel_spmd

F32 = mybir.dt.float32
F32R = mybir.dt.float32r
BF16 = mybir.dt.bfloat16
AFT = mybir.ActivationFunctionType
BF = ml_dtypes.bfloat16

D = 1024          # model dim
T = 2048          # sequence length
B = 2             # batch
HEADS = 16        # total heads
DK = 64           # head dim
NCORES = 8
GH = 4            # heads per core
GD = GH * DK      # 256 projection cols per core
NF = D // 128     # 8 contraction chunks
NKT = T // 128    # 16 k tiles
NQS = 4           # 512-wide q stripes
QW = T // NQS     # 512
SCALE = 1.0 / np.sqrt(np.float32(DK))  # 1/8

# Results of the last run (for test harness introspection: exec_time_ns etc.)
LAST_RESULTS = None


@functools.lru_cache(maxsize=1)
def _build_program():
    nc = bacc.Bacc("TRN2", target_bir_lowering=False, debug=False,
                   num_devices=NCORES)

    # host-packed activation layouts (see _pack_* in kernel()):
    #   xq[s]  = [128, NF*QW]  q-stripe s, chunk-major (8 KiB DMA lines)
    #   xk[qh] = [128, NF*QW]  k column-group qh, chunk-major
    #   xv[tb] = [128, NF*128] k-tile tb, chunk-major (2 KiB lines)
    xq = nc.declare_dram_parameter("xq", [NQS, 128, NF * QW], BF16,
                                   isOutput=False)
    xk = nc.declare_dram_parameter("xk", [4, 128, NF * QW], BF16,
                                   isOutput=False)
    xv = nc.declare_dram_parameter("xv", [NKT, 128, NF * 128], BF16,
                                   isOutput=False)
    wq = nc.declare_dram_parameter("wq", [128, NF * GD], BF16, isOutput=False)
    wk = nc.declare_dram_parameter("wk", [128, NF * GD], BF16, isOutput=False)
    wv = nc.declare_dram_parameter("wv", [128, NF * GD], BF16, isOutput=False)
    wo = nc.declare_dram_parameter("wo", [2, 128, D], BF16, isOutput=False)
    bqv = nc.declare_dram_parameter("bqv", [128, 2], F32, isOutput=False)
    out = nc.declare_dram_parameter("out", [T, D], BF16, isOutput=True)

    import contextlib
    with tile.TileContext(nc) as tc, contextlib.ExitStack() as _st:
        # ---- persistent pools -------------------------------------------
        def _pool(**kw):
            return _st.enter_context(tc.tile_pool(**kw))

        if True:
            kt_pool = _pool(name="kt", bufs=2)
            vext_pool = _pool(name="vext", bufs=NKT)
            qts_pool = _pool(name="qts", bufs=NQS * 2)
            ots_pool = _pool(name="ots", bufs=NQS * 2)
            w_pool = _pool(name="wts", bufs=3)
            wo_pool = _pool(name="wop", bufs=2)
            xq_pool = _pool(name="xq", bufs=NQS)
            xk_pool = _pool(name="xk", bufs=4)
            xv_pool = _pool(name="xv", bufs=NKT)
            const_pool = _pool(name="const", bufs=1)
            ones_f32 = const_pool.tile([128, GH], F32, tag="ones32")
            nc.gpsimd.memset(ones_f32[:], 1.0)
            ones_bf = const_pool.tile([128, DK], BF16, tag="onesbf")
            nc.gpsimd.memset(ones_bf[:], 1.0)
            # head-pair selectors: sel[hp][c, m] = (c == 32*(hp*2 + m//64)),
            # i.e. r_bc[m, :] = rinvT[32*head(m), :] after the C=128 matmul
            sel = [const_pool.tile([128, 128], BF16, tag=f"sel{hp}",
                                   name=f"sel{hp}") for hp in range(2)]
            for hp in range(2):
                nc.gpsimd.memset(sel[hp][:], 0.0)
                for hh in range(2):
                    c = 32 * (hp * 2 + hh)
                    nc.vector.tensor_copy(
                        sel[hp][c:c + 1, hh * DK:(hh + 1) * DK],
                        ones_bf[0:1, 0:DK])
            bqv_sb = const_pool.tile([128, 2], F32, tag="bqv")
            nc.sync.dma_start(bqv_sb[:], bqv[:])

            KT = [kt_pool.tile([128, T], BF16, tag="kt", name=f"kt{m}")
                  for m in range(2)]
            VE = [vext_pool.tile([128, GH * (DK + 1)], BF16, tag="vext",
                                 name=f"ve{i}") for i in range(NKT)]
            # per-stripe Q^T and O^T tiles (heads of pair hp stacked 64+64)
            QTs = [[qts_pool.tile([128, QW], BF16, tag="qts",
                                  name=f"qt{s}_{m}") for m in range(2)]
                   for s in range(NQS)]
            OTs = [[ots_pool.tile([128, QW], BF16, tag="ots",
                                  name=f"ot{s}_{m}") for m in range(2)]
                   for s in range(NQS)]
            WO = [wo_pool.tile([128, D], BF16, tag="wop", name=f"wo{m}")
                  for m in range(2)]

            wq_sb = w_pool.tile([128, NF * GD], BF16, tag="w", name="wq_sb")
            wk_sb = w_pool.tile([128, NF * GD], BF16, tag="w", name="wk_sb")
            wv_sb = w_pool.tile([128, NF * GD], BF16, tag="w", name="wv_sb")
            XQs = [xq_pool.tile([128, NF * QW], BF16, tag="xq",
                                name=f"xqs{s}") for s in range(NQS)]
            XKq = [xk_pool.tile([128, NF * QW], BF16, tag="xk",
                                name=f"xkq{i}") for i in range(4)]
            XVt = [xv_pool.tile([128, NF * 128], BF16, tag="xv",
                                name=f"xvt{i}") for i in range(NKT)]

            # V_ext ones columns (persistent; written once, no DMA dep)
            for tb in range(NKT):
                ve_r = VE[tb][:].rearrange("p (h x) -> p h x", x=DK + 1)
                nc.vector.tensor_copy(
                    ve_r[:, :, DK:DK + 1],
                    ones_f32[:].rearrange("p (h x) -> p h x", x=1))

            # ---- DMA in, ordered to match the projection filler schedule
            # (upfront: Q stripe 0, K group 0, V tiles 0-5; the rest lands
            # under stripe 0's attention).  All transfers stay on the sync
            # engine's HWDGE queue: routing part of them through the
            # scalar engine's queue was tried and regressed ~10us (the
            # descriptor issue traffic perturbs the exp stream).
            nc.sync.dma_start(wq_sb[:], wq[:])
            nc.sync.dma_start(XQs[0][:], xq[0])
            nc.sync.dma_start(wk_sb[:], wk[:])
            nc.sync.dma_start(XKq[0][:], xk[0])
            nc.sync.dma_start(wv_sb[:], wv[:])
            for tb in range(6):
                nc.sync.dma_start(XVt[tb][:], xv[tb])
            nc.sync.dma_start(XKq[1][:], xk[1])
            for tb in range(6, 10):
                nc.sync.dma_start(XVt[tb][:], xv[tb])
            nc.sync.dma_start(XKq[2][:], xk[2])
            nc.sync.dma_start(XKq[3][:], xk[3])
            for tb in range(10, NKT):
                nc.sync.dma_start(XVt[tb][:], xv[tb])
            for s in range(1, NQS):
                nc.sync.dma_start(XQs[s][:], xq[s])
            nc.sync.dma_start(WO[0][:], wo[0])
            nc.sync.dma_start(WO[1][:], wo[1])

            # (A same-bank row-tiled pair variant of these projections --
            # start=False accumulation onto a pre-zeroed bank -- was
            # tried and hangs the device at runtime; keep full-C MMs.)
            def q_project(s, m, ps_q, fc):
                nc.tensor.matmul(
                    ps_q[:],
                    wq_sb[:, fc * GD + m * 128:fc * GD + (m + 1) * 128],
                    XQs[s][:, fc * QW:(fc + 1) * QW],
                    start=(fc == 0), stop=(fc == NF - 1))
                if fc == NF - 1:
                    nc.vector.tensor_scalar_add(
                        QTs[s][m][:], ps_q[:], bqv_sb[:, m:m + 1])

            def k_project(qh, m, ps_k, fc):
                nc.tensor.matmul(
                    ps_k[:],
                    wk_sb[:, fc * GD + m * 128:fc * GD + (m + 1) * 128],
                    XKq[qh][:, fc * QW:(fc + 1) * QW],
                    start=(fc == 0), stop=(fc == NF - 1))
                if fc == NF - 1:
                    nc.vector.tensor_copy(
                        KT[m][:, qh * QW:(qh + 1) * QW], ps_k[:])

            def v_project(tb, ps_v, dc):
                nc.tensor.matmul(
                    ps_v[:, 0:GD],
                    XVt[tb][:, dc * 128:(dc + 1) * 128],
                    wv_sb[:, dc * GD:(dc + 1) * GD],
                    start=(dc == 0), stop=(dc == NF - 1))
                if dc == NF - 1:
                    ve_r = VE[tb][:].rearrange("p (h x) -> p h x", x=DK + 1)
                    nc.vector.tensor_copy(
                        ve_r[:, :, 0:DK],
                        ps_v[:, 0:GD].rearrange("p (h x) -> p h x", x=DK))

            # ---- phase A: warm the HAM clock gate with dummy matmuls
            # (no DMA dependency), then project Q stripe 0, K group 0 and
            # V tiles 0-5 as their inputs land.
            with tc.tile_pool(name="psA", bufs=8,
                              space=bass.MemorySpace.PSUM) as psA:
                # ~8.5us of dependency-free matmuls: warms the HAM clock
                # gate AND covers the ~7us DMA/preamble startup so real
                # projections start the moment their data lands.
                warm = psA.tile([128, QW], F32, tag="psA", name="warm")
                for i in range(150):
                    nc.tensor.matmul(
                        warm[0:DK, 0:DK], ones_bf[:, 0:DK],
                        ones_bf[:, 0:DK], start=True, stop=True)
                for m in range(2):
                    ps_q = psA.tile([128, QW], F32, tag="psA", name=f"psq{m}")
                    for fc in range(NF):
                        q_project(0, m, ps_q, fc)
                for m in range(2):
                    ps_k = psA.tile([128, QW], F32, tag="psA",
                                    name=f"psk0_{m}")
                    for fc in range(NF):
                        k_project(0, m, ps_k, fc)
                for tb in range(6):
                    ps_v = psA.tile([128, QW], F32, tag="psA",
                                    name=f"psv{tb}")
                    for dc in range(NF):
                        v_project(tb, ps_v, dc)

            # ---- phase B: striped attention with PE fillers -------------
            with contextlib.ExitStack() as _stB:
                def _poolB(**kw):
                    return _stB.enter_context(tc.tile_pool(**kw))

                es_pool = _poolB(name="ep", bufs=5)
                ub_pool = _poolB(name="ubp", bufs=8)
                rs_pool = _poolB(name="rsp", bufs=2)
                ob_pool = _poolB(name="obp", bufs=4)
                psS = _poolB(name="psS", bufs=2,
                             space=bass.MemorySpace.PSUM)
                psO = _poolB(name="psO", bufs=2,
                             space=bass.MemorySpace.PSUM)
                psF = _poolB(name="psF", bufs=2,
                             space=bass.MemorySpace.PSUM)
                ub_tiles = {}     # (qs, hp, hh) -> [64, 512] f32 tile
                rs_tiles = {}     # qs -> [128, 512] f32 rowsum-spread tile

                fstate = {}

                def qproj_fillers(s):
                    fs = []
                    for m in range(2):
                        def mk(mm, fc):
                            def f():
                                if fc == 0:
                                    fstate['q', mm] = psF.tile(
                                        [128, QW], F32, tag="psF",
                                        name=f"psq{s}_{mm}")
                                q_project(s, mm, fstate['q', mm], fc)
                            return f
                        for fc in range(NF):
                            fs.append(mk(m, fc))
                    return fs

                def kq_fillers(qh, m):
                    """K projection of column-group qh, head-pair tile m
                    (2 chunk-pairs per filler)."""
                    def mk(fp):
                        def f():
                            if fp == 0:
                                fstate['k', qh, m] = psF.tile(
                                    [128, QW], F32, tag="psF",
                                    name=f"psk{qh}_{m}")
                            for fc in (2 * fp, 2 * fp + 1):
                                k_project(qh, m, fstate['k', qh, m], fc)
                        return f
                    return [mk(fp) for fp in range(4)]

                def vtb_fillers(tb):
                    """V projection of k-tile tb (4 chunk-pairs/filler)."""
                    def mk(dp):
                        def f():
                            if dp == 0:
                                fstate['v', tb] = psF.tile(
                                    [128, QW], F32, tag="psF",
                                    name=f"psv{tb}")
                            for dc in range(4 * dp, 4 * dp + 4):
                                v_project(tb, fstate['v', tb], dc)
                        return f
                    return [mk(0), mk(1)]

                def recip_fillers(s, hps=(0, 1), state={}):
                    """Reciprocal dance + normalize for stripe s (rowsums
                    already collected at partitions 32h of rs_tiles[s]).
                    Split into [transpose, ln/exp/transpose, bcast...] so
                    the caller can space the ACT work away from its DVE
                    dependency in the filler stream."""
                    fs = []

                    def t1():
                        rsT = rs_pool.tile([128, QW], F32, tag="rsT",
                                           name=f"rsT{s}")
                        nc.vector.transpose(rsT[:], rs_tiles[s][:])
                        state[s] = rsT
                    fs.append(t1)

                    def t2():
                        rsT = state.pop(s)
                        nc.scalar.activation(rsT[:], rsT[:], AFT.Ln)
                        rinv = rs_pool.tile([128, QW], BF16, tag="rinv",
                                            name=f"rinv{s}")
                        nc.scalar.activation(rinv[:], rsT[:],
                                             AFT.Exp, scale=-1.0)
                        rinvT = rs_pool.tile([128, QW], BF16, tag="rinvT",
                                             name=f"rinvT{s}")
                        nc.vector.transpose(rinvT[:], rinv[:])
                        recip_fillers.rinvT = rinvT
                    fs.append(t2)

                    def mk_bcast(hp):
                        def f():
                            r_bc = psF.tile([128, QW], F32, tag="psF",
                                            name=f"rbc{s}_{hp}")
                            nc.tensor.matmul(
                                r_bc[:],
                                sel[hp][:],
                                recip_fillers.rinvT[:],
                                start=True, stop=True)
                            for hh in range(2):
                                nc.vector.tensor_mul(
                                    OTs[s][hp][hh * DK:(hh + 1) * DK, :],
                                    ub_tiles.pop((s, hp, hh))[0:DK, :],
                                    r_bc[hh * DK:(hh + 1) * DK, :])
                        return f
                    for hp in hps:
                        fs.append(mk_bcast(hp))
                    return fs
                recip_fillers.rinvT = None

                def outproj_fillers(s):
                    """Each (tt, ei) unit is split into a matmul closure
                    and a drain closure so the PE filler bursts stay
                    fine-grained inside the exp-bound attention cadence."""
                    fs = []

                    def mk_mm(tt, ei):
                        def f():
                            if ei == 0:
                                outproj_fillers.ob = ob_pool.tile(
                                    [128, D], BF16, tag="ob",
                                    name=f"ob{s}_{tt}")
                            f_ps = psF.tile([128, QW], F32, tag="psF",
                                            name=f"fps{s}_{tt}_{ei}")
                            fstate['op'] = f_ps
                            for m in range(2):
                                nc.tensor.matmul(
                                    f_ps[:],
                                    OTs[s][m][:, tt * 128:(tt + 1) * 128],
                                    WO[m][:, ei * QW:(ei + 1) * QW],
                                    start=(m == 0), stop=(m == 1))
                        return f

                    def mk_drain(tt, ei):
                        def f():
                            ob = outproj_fillers.ob
                            f_ps = fstate.pop('op')
                            if s == NQS - 1 and (tt + ei) % 2 == 0:
                                # tail: alternate drains between the idle
                                # ACT queue and DVE so they run 2-wide
                                nc.scalar.activation(
                                    ob[:, ei * QW:(ei + 1) * QW], f_ps[:],
                                    AFT.Copy)
                            else:
                                nc.vector.tensor_copy(
                                    ob[:, ei * QW:(ei + 1) * QW], f_ps[:])
                            if ei == 1:
                                t0 = (s * 4 + tt) * 128
                                nc.sync.dma_start(out[t0:t0 + 128, :], ob[:])
                        return f
                    for tt in range(4):
                        for ei in range(2):
                            fs.append(mk_mm(tt, ei))
                            fs.append(mk_drain(tt, ei))
                    return fs
                outproj_fillers.ob = None

                # flat (qs, hp, kt) stream: aV is emitted 1-3 steps behind
                # scores/exp so the FIFO PE queue never waits on an exp
                # before issuing independent scores work.  At block starts
                # the hold-back deepens to 3 so the previous block's DVE
                # drains (which gate aV(kt0) via o_ps buffer reuse) finish
                # under the run-ahead scores instead of stalling the PE.
                fillers = deque()
                pending = deque()  # (qs, hp, o_ps, es, kt)

                def flush_one():
                    pqs, php, po_ps, pes, pkt = pending.popleft()
                    for hh in range(2):
                        h = php * 2 + hh
                        nc.tensor.matmul(
                            po_ps[hh][0:DK + 1, :],
                            VE[pkt][:, h * (DK + 1):(h + 1) * (DK + 1)],
                            pes[:, hh * QW:(hh + 1) * QW],
                            start=(pkt == 0), stop=(pkt == NKT - 1))
                    if pkt == NKT - 1:
                        # drain O^T + rowsum row; heads at partitions 32h.
                        # The very last block's drains go on the otherwise
                        # idle ACT queue to shorten the serial tail.
                        last = pqs == NQS - 1 and php == 1
                        for hh in range(2):
                            h = php * 2 + hh
                            u = ub_pool.tile([128, QW], F32, tag="ub",
                                             name=f"ub{pqs}_{php}_{hh}")
                            if last and hh == 1:
                                nc.scalar.activation(
                                    u[0:DK, :], po_ps[hh][0:DK, :],
                                    AFT.Copy)
                                nc.scalar.activation(
                                    rs_tiles[pqs][32 * h:32 * h + 1, :],
                                    po_ps[hh][DK:DK + 1, :], AFT.Copy)
                            else:
                                nc.vector.tensor_copy(
                                    u[0:DK, :], po_ps[hh][0:DK, :])
                                nc.vector.tensor_copy(
                                    rs_tiles[pqs][32 * h:32 * h + 1, :],
                                    po_ps[hh][DK:DK + 1, :])
                            ub_tiles[(pqs, php, hh)] = u
                        if pqs == NQS - 1 and php == 0:
                            # last stripe: overlap hp0's half of the
                            # reciprocal dance under hp1's attention
                            rf = recip_fillers(pqs, hps=(0,))
                            fillers.append(rf[0])
                            fillers.extend([spacer] * 3)
                            fillers.extend(rf[1:])

                def spacer():
                    pass

                for qs in range(NQS):
                    rf = recip_fillers(qs - 1) if qs > 0 else []
                    qp = qproj_fillers(qs + 1) if qs < NQS - 1 else []
                    if qs == 0:
                        # remaining input projections ride along stripe 0
                        # (2 filler pops per kt), ordered so every tile's
                        # drain is emitted before its first consumer
                        fillers.extend(kq_fillers(1, 0))
                        for tb in range(6, 10):
                            fillers.extend(vtb_fillers(tb))
                        fillers.extend(kq_fillers(2, 0))
                        fillers.extend(kq_fillers(3, 0))
                        for tb in range(10, NKT):
                            fillers.extend(vtb_fillers(tb))
                        for qh in range(1, 4):
                            fillers.extend(kq_fillers(qh, 1))
                        fillers.extend(qp)
                    elif rf:
                        fillers.append(rf[0])       # DVE transpose
                        if qp:
                            fillers.extend(qp[0:8])  # qproj m0 (pins psF)
                        else:
                            fillers.extend([spacer] * 4)
                        fillers.extend(rf[1:])      # Ln/Exp + bcasts
                        fillers.extend(outproj_fillers(qs - 1))
                        fillers.extend(qp[8:16])    # qproj m1
                    else:
                        fillers.extend(qp)

                    rs_t = rs_pool.tile([128, QW], F32, tag="rs",
                                        name=f"rs{qs}")
                    nc.gpsimd.memset(rs_t[:], 1.0)
                    rs_tiles[qs] = rs_t

                    for hp in range(2):
                        o_ps = [psO.tile([128, QW], F32, tag="psO",
                                         name=f"o{qs}_{hp}_{i}")
                                for i in range(2)]
                        for kt in range(NKT):
                            sc = psS.tile([128, 2 * QW], F32, tag="psS",
                                          name=f"s{qs}_{hp}_{kt}")
                            for hh in range(2):
                                lo = hh * DK
                                nc.tensor.matmul(
                                    sc[:, hh * QW:(hh + 1) * QW],
                                    KT[hp][lo:lo + DK,
                                           kt * 128:(kt + 1) * 128],
                                    QTs[qs][hp][lo:lo + DK, :],
                                    start=True, stop=True)
                            es = es_pool.tile([128, 2 * QW], BF16, tag="es",
                                              name=f"e{qs}_{hp}_{kt}")
                            nc.scalar.activation(es[:], sc[:], AFT.Exp,
                                                 scale=float(SCALE))
                            # flush older blocks now; hold up to 3 of the
                            # current block while kt < 3
                            while pending and pending[0][0:2] != (qs, hp):
                                flush_one()
                            pending.append((qs, hp, o_ps, es, kt))
                            target = 3 if kt < 3 else (2 if kt < 5 else 1)
                            while len(pending) > target:
                                flush_one()
                            # NOTE: draining leftover fillers faster near
                            # the stripe seam (2 pops over the last kts)
                            # was tried and regressed 211us -> 252us; the
                            # single-pop cadence is load-bearing.
                            for _ in range(2 if qs == 0 else 1):
                                if fillers:
                                    fillers.popleft()()
                    # leftover fillers must land before the next stripe's
                    # scores read tiles they write (QTs of qs+1)
                    while fillers:
                        fillers.popleft()()

                # tail: flush last aV + drains, hp1 dance, outproj.
                # ~4.3us of dependency-free matmuls span the PE-idle
                # reciprocal-dance window so the HAM clock gate stays at
                # 2.4 GHz for the final output-projection matmuls.
                while pending:
                    flush_one()
                warm2 = psF.tile([128, QW], F32, tag="psF", name="warm2")
                for i in range(20):
                    nc.tensor.matmul(
                        warm2[0:DK, :], ones_bf[:, 0:DK], KT[0][:, 0:QW],
                        start=True, stop=True)
                for f in recip_fillers(NQS - 1, hps=(1,)):
                    f()
                for f in outproj_fillers(NQS - 1):
                    f()

    from concourse.bacc import get_activation_tables
    import bass_rust as _br
    _combined = "natural_log_exp_and_others"
    _tabs = []
    for _name, _fns in get_activation_tables(nc.m.arch).items():
        if _name != _combined:
            _fns = _fns - {AFT.Exp, AFT.Ln}
        _tabs.append((_name, _fns))
    _br.insert_act_table_loads(nc, _tabs)
    nc.compile()
    return nc


def _numpy_reference(q, k, v, mask, Wq, bq, Wk, bk, Wv, bv, Wo, bo):
    """Fallback for a non-trivial mask (never hit with the stock inputs)."""
    Bn, Tn, _ = q.shape
    H, dk = HEADS, DK

    def split(x):
        return x.reshape(Bn, Tn, H, dk).transpose(0, 2, 1, 3)

    qh = split(q @ Wq + bq)
    kh = split(k @ Wk + bk)
    vh = split(v @ Wv + bv)
    s = np.einsum("bhqd,bhkd->bhqk", qh, kh) / np.sqrt(np.float32(dk))
    s = np.where(mask, s, -np.inf)
    s = s - s.max(axis=-1, keepdims=True)
    e = np.exp(s)
    a = e / e.sum(axis=-1, keepdims=True)
    o = np.einsum("bhqk,bhkd->bhqd", a, vh)
    o = o.transpose(0, 2, 1, 3).reshape(Bn, Tn, H * dk)
    return (o @ Wo + bo).astype(np.float32)


def kernel(q, k, v, mask, Wq, bq, Wk, bk, Wv, bv, Wo, bo):
    global LAST_RESULTS
    q = np.asarray(q, np.float32)
    k = np.asarray(k, np.float32)
    v = np.asarray(v, np.float32)
    mask = np.asarray(mask, bool)
    Wq, bq = np.asarray(Wq, np.float32), np.asarray(bq, np.float32)
    Wk, bk = np.asarray(Wk, np.float32), np.asarray(bk, np.float32)
    Wv, bv = np.asarray(Wv, np.float32), np.asarray(bv, np.float32)
    Wo, bo = np.asarray(Wo, np.float32), np.asarray(bo, np.float32)

    if not mask.all():
        return _numpy_reference(q, k, v, mask, Wq, bq, Wk, bk, Wv, bv, Wo, bo)

    nc = _build_program()

    # host-side sharding; activations packed chunk-major per column
    # group (see the dram parameter comments in _build_program)
    def pack_cols(xT_b, w):
        ng = T // w
        return np.ascontiguousarray(
            xT_b.reshape(NF, 128, ng, w).transpose(2, 1, 0, 3)
            .reshape(ng, 128, NF * w))

    xP = {}
    for b in range(B):
        xq_t, xk_t, xv_t = (x[b].T.astype(BF) for x in (q, k, v))
        xP[b] = (pack_cols(xq_t, QW), pack_cols(xk_t, QW),
                 pack_cols(xv_t, 128))

    def w_chunks(W, g):
        # (1024, 256) head-group slice -> [128, 8*256] chunk-major layout
        Wg = W[:, g * GD:(g + 1) * GD]
        return np.ascontiguousarray(
            Wg.reshape(NF, 128, GD).transpose(1, 0, 2)
            .reshape(128, NF * GD).astype(BF))

    in_maps = []
    for c in range(NCORES):
        b, g = divmod(c, GH)
        xq_t, xk_t, xv_t = xP[b]
        in_maps.append({
            "xq": xq_t, "xk": xk_t, "xv": xv_t,
            "wq": w_chunks(Wq, g), "wk": w_chunks(Wk, g),
            "wv": w_chunks(Wv, g),
            "wo": np.ascontiguousarray(
                Wo[g * GD:(g + 1) * GD, :].astype(BF)).reshape(2, 128, D),
            "bqv": np.ascontiguousarray(
                bq[g * GD:(g + 1) * GD].reshape(2, 128).T),
        })

    LAST_RESULTS = run_bass_kernel_spmd(
        nc, in_maps, list(range(NCORES)),
        trace=bool(os.environ.get("KERNEL_TRACE")))
    res = LAST_RESULTS.results

    const_row = (bv @ Wo + bo).astype(np.float32)  # attn rows sum to 1
    full = np.empty((B, T, D), np.float32)
    for b in range(B):
        acc = res[b * GH]["out"].astype(np.float32)
        for g in range(1, GH):
            acc = acc + res[b * GH + g]["out"].astype(np.float32)
        full[b] = acc + const_row
    return full


# revision 71
# speedup vs baseline: 1.2014x; 1.2014x over previous
"""Trainium2 Bass kernel: 16-head MHA (B=2, T=2048, D=1024, d_k=64).

Sharding (8 NeuronCores): data-parallel over the batch (2) x tensor-parallel
over head groups (4 groups of 4 heads).  Core c handles batch b = c//4 and
heads [4g, 4g+4) with g = c%4.  Each core computes its partial output
    sum_{h in group} softmax((q Wq_h + bq_h)(k Wk_h)^T / 8) (v Wv_h) Wo_h
and the host sums the 4 partials per batch and adds the constant row
bo + bv @ Wo once.  bk is dropped: with the all-ones mask it shifts every
score row by a per-row constant, which softmax ignores exactly.

Design notes (420us baseline -> ~211us):
  * every matmul operand is bf16 (FWL weight loads, fp32 PSUM
    accumulate); output DMA'd as bf16 and upconverted host-side.
    The two heads of a pair sit on partition halves 0:64 / 64:128, so
    their C=64 scores matmuls land on disjoint PE row-tiles (T0/T8) and
    execute CONCURRENTLY (measured: starts 3 ns apart) -- scores cost
    half the naive streaming time.  (Column-tiled pair splits of the
    C=128 projections were tried and serialize; only row tiles overlap.)
  * V is projected directly in [t, v-col] layout (stationary = x^T
    chunk, moving = Wv) -- no PE transposes.  Activations arrive via
    host-packed chunk-major layouts (xq/xk per 512-col group, xv per
    128-row k-tile) so each projection unit depends on ~1 MB of DMA,
    not the whole tensor.
  * attention runs in 512-wide q stripes; per (stripe, head-pair, kt):
    2 concurrent scores MMs -> one [128,1024] fp32 PSUM tile, one ACT
    exp -> bf16 es, 2 aV MMs accumulating into per-head [65,512] PSUM
    (the 65th V_ext ones-column yields softmax rowsums for free, which
    is column-optimal: a separate rowsum matmul would stream es again).
    PSUM: scores dbuf 2x2 banks + O-accum 2 + filler 2 = 8 exactly.
  * one flat software-pipelined (stripe, hp, kt) stream: aV runs 1-3
    steps behind scores/exp (deepened at block starts) so the FIFO PE
    queue never waits on an exp or on the previous block's drains.
  * the PE is kept dense -- and the HAM clock gate at 2.4 GHz -- by
    emitting everything else as fillers inside the ACT-bound attention
    loop: K groups 1-3 + V tiles 6-15 + Q stripe s+1 projections, the
    reciprocal dance, and stripe s-1's output projection, each unit
    ordered so its drain is emitted before its first consumer.
    ~150 dependency-free warm-up matmuls cover the ~7us DMA startup.
  * softmax denominators: rowsum rows are collected at partitions 32h,
    DVE 32-block transposed, one batched ACT Ln + Exp(-1) computes all
    reciprocals partition-parallel, transposed back, and broadcast
    across partitions by one C=128 selector matmul per head pair.
    The last stripe runs hp0's dance under hp1's attention and drains
    on the otherwise-idle ACT queue to shorten the serial tail.
"""

import functools
import os
from collections import deque

import ml_dtypes
import numpy as np

import concourse.bass as bass
import concourse.mybir as mybir
import concourse.tile as tile
from concourse import bacc
from concourse.bass_utils import run_bass_kernel_spmd

F32 = mybir.dt.float32
F32R = mybir.dt.float32r
BF16 = mybir.dt.bfloat16
AFT = mybir.ActivationFunctionType
BF = ml_dtypes.bfloat16

D = 1024          # model dim
T = 2048          # sequence length
B = 2             # batch
HEADS = 16        # total heads
DK = 64           # head dim
NCORES = 8
GH = 4            # heads per core
GD = GH * DK      # 256 projection cols per core
NF = D // 128     # 8 contraction chunks
NKT = T // 128    # 16 k tiles
NQS = 4           # 512-wide q stripes
QW = T // NQS     # 512
SCALE = 1.0 / np.sqrt(np.float32(DK))  # 1/8

# Results of the last run (for test harness introspection: exec_time_ns etc.)
LAST_RESULTS = None


@functools.lru_cache(maxsize=1)
def _build_program():
    nc = bacc.Bacc("TRN2", target_bir_lowering=False, debug=False,
                   num_devices=NCORES)

    # host-packed activation layouts (see _pack_* in kernel()):
    #   xq[s]  = [128, NF*QW]  q-stripe s, chunk-major (8 KiB DMA lines)
    #   xk[qh] = [128, NF*QW]  k column-group qh, chunk-major
    #   xv[tb] = [128, NF*128] k-tile tb, chunk-major (2 KiB lines)
    xq = nc.declare_dram_parameter("xq", [NQS, 128, NF * QW], BF16,
                                   isOutput=False)
    xk = nc.declare_dram_parameter("xk", [4, 128, NF * QW], BF16,
                                   isOutput=False)
    xv = nc.declare_dram_parameter("xv", [NKT, 128, NF * 128], BF16,
                                   isOutput=False)
    wq = nc.declare_dram_parameter("wq", [128, NF * GD], BF16, isOutput=False)
    wk = nc.declare_dram_parameter("wk", [128, NF * GD], BF16, isOutput=False)
    wv = nc.declare_dram_parameter("wv", [128, NF * GD], BF16, isOutput=False)
    wo = nc.declare_dram_parameter("wo", [2, 128, D], BF16, isOutput=False)
    bqv = nc.declare_dram_parameter("bqv", [128, 2], F32, isOutput=False)
    out = nc.declare_dram_parameter("out", [T, D], BF16, isOutput=True)

    import contextlib
    with tile.TileContext(nc) as tc, contextlib.ExitStack() as _st:
        # ---- persistent pools -------------------------------------------
        def _pool(**kw):
            return _st.enter_context(tc.tile_pool(**kw))

        if True:
            kt_pool = _pool(name="kt", bufs=2)
            vext_pool = _pool(name="vext", bufs=NKT)
            qts_pool = _pool(name="qts", bufs=NQS * 2)
            ots_pool = _pool(name="ots", bufs=NQS * 2)
            w_pool = _pool(name="wts", bufs=3)
            wo_pool = _pool(name="wop", bufs=2)
            xq_pool = _pool(name="xq", bufs=NQS)
            xk_pool = _pool(name="xk", bufs=4)
            xv_pool = _pool(name="xv", bufs=NKT)
            const_pool = _pool(name="const", bufs=1)
            ones_f32 = const_pool.tile([128, GH], F32, tag="ones32")
            nc.gpsimd.memset(ones_f32[:], 1.0)
            ones_bf = const_pool.tile([128, DK], BF16, tag="onesbf")
            nc.gpsimd.memset(ones_bf[:], 1.0)
            # head-pair selectors: sel[hp][c, m] = (c == 32*(hp*2 + m//64)),
            # i.e. r_bc[m, :] = rinvT[32*head(m), :] after the C=128 matmul
            sel = [const_pool.tile([128, 128], BF16, tag=f"sel{hp}",
                                   name=f"sel{hp}") for hp in range(2)]
            for hp in range(2):
                nc.gpsimd.memset(sel[hp][:], 0.0)
                for hh in range(2):
                    c = 32 * (hp * 2 + hh)
                    nc.vector.tensor_copy(
                        sel[hp][c:c + 1, hh * DK:(hh + 1) * DK],
                        ones_bf[0:1, 0:DK])
            bqv_sb = const_pool.tile([128, 2], F32, tag="bqv")
            nc.sync.dma_start(bqv_sb[:], bqv[:])

            KT = [kt_pool.tile([128, T], BF16, tag="kt", name=f"kt{m}")
                  for m in range(2)]
            VE = [vext_pool.tile([128, GH * (DK + 1)], BF16, tag="vext",
                                 name=f"ve{i}") for i in range(NKT)]
            # per-stripe Q^T and O^T tiles (heads of pair hp stacked 64+64)
            QTs = [[qts_pool.tile([128, QW], BF16, tag="qts",
                                  name=f"qt{s}_{m}") for m in range(2)]
                   for s in range(NQS)]
            OTs = [[ots_pool.tile([128, QW], BF16, tag="ots",
                                  name=f"ot{s}_{m}") for m in range(2)]
                   for s in range(NQS)]
            WO = [wo_pool.tile([128, D], BF16, tag="wop", name=f"wo{m}")
                  for m in range(2)]

            wq_sb = w_pool.tile([128, NF * GD], BF16, tag="w", name="wq_sb")
            wk_sb = w_pool.tile([128, NF * GD], BF16, tag="w", name="wk_sb")
            wv_sb = w_pool.tile([128, NF * GD], BF16, tag="w", name="wv_sb")
            XQs = [xq_pool.tile([128, NF * QW], BF16, tag="xq",
                                name=f"xqs{s}") for s in range(NQS)]
            XKq = [xk_pool.tile([128, NF * QW], BF16, tag="xk",
                                name=f"xkq{i}") for i in range(4)]
            XVt = [xv_pool.tile([128, NF * 128], BF16, tag="xv",
                                name=f"xvt{i}") for i in range(NKT)]

            # V_ext ones columns (persistent; written once, no DMA dep)
            for tb in range(NKT):
                ve_r = VE[tb][:].rearrange("p (h x) -> p h x", x=DK + 1)
                nc.vector.tensor_copy(
                    ve_r[:, :, DK:DK + 1],
                    ones_f32[:].rearrange("p (h x) -> p h x", x=1))

            # ---- DMA in, ordered to match the projection filler schedule
            # (upfront: Q stripe 0, K group 0, V tiles 0-5; the rest lands
            # under stripe 0's attention).  All transfers stay on the sync
            # engine's HWDGE queue: routing part of them through the
            # scalar engine's queue was tried and regressed ~10us (the
            # descriptor issue traffic perturbs the exp stream).
            nc.sync.dma_start(wq_sb[:], wq[:])
            nc.sync.dma_start(XQs[0][:], xq[0])
            nc.sync.dma_start(wk_sb[:], wk[:])
            nc.sync.dma_start(XKq[0][:], xk[0])
            nc.sync.dma_start(wv_sb[:], wv[:])
            for tb in range(6):
                nc.sync.dma_start(XVt[tb][:], xv[tb])
            nc.sync.dma_start(XKq[1][:], xk[1])
            for tb in range(6, 10):
                nc.sync.dma_start(XVt[tb][:], xv[tb])
            nc.sync.dma_start(XKq[2][:], xk[2])
            nc.sync.dma_start(XKq[3][:], xk[3])
            for tb in range(10, NKT):
                nc.sync.dma_start(XVt[tb][:], xv[tb])
            for s in range(1, NQS):
                nc.sync.dma_start(XQs[s][:], xq[s])
            nc.sync.dma_start(WO[0][:], wo[0])
            nc.sync.dma_start(WO[1][:], wo[1])

            # (A same-bank row-tiled pair variant of these projections --
            # start=False accumulation onto a pre-zeroed bank -- was
            # tried and hangs the device at runtime; keep full-C MMs.)
            def q_project(s, m, ps_q, fc):
                nc.tensor.matmul(
                    ps_q[:],
                    wq_sb[:, fc * GD + m * 128:fc * GD + (m + 1) * 128],
                    XQs[s][:, fc * QW:(fc + 1) * QW],
                    start=(fc == 0), stop=(fc == NF - 1))
                if fc == NF - 1:
                    nc.vector.tensor_scalar_add(
                        QTs[s][m][:], ps_q[:], bqv_sb[:, m:m + 1])

            def k_project(qh, m, ps_k, fc):
                nc.tensor.matmul(
                    ps_k[:],
                    wk_sb[:, fc * GD + m * 128:fc * GD + (m + 1) * 128],
                    XKq[qh][:, fc * QW:(fc + 1) * QW],
                    start=(fc == 0), stop=(fc == NF - 1))
                if fc == NF - 1:
                    nc.vector.tensor_copy(
                        KT[m][:, qh * QW:(qh + 1) * QW], ps_k[:])

            def v_project(tb, ps_v, dc):
                nc.tensor.matmul(
                    ps_v[:, 0:GD],
                    XVt[tb][:, dc * 128:(dc + 1) * 128],
                    wv_sb[:, dc * GD:(dc + 1) * GD],
                    start=(dc == 0), stop=(dc == NF - 1))
                if dc == NF - 1:
                    ve_r = VE[tb][:].rearrange("p (h x) -> p h x", x=DK + 1)
                    nc.vector.tensor_copy(
                        ve_r[:, :, 0:DK],
                        ps_v[:, 0:GD].rearrange("p (h x) -> p h x", x=DK))

            # ---- phase A: warm the HAM clock gate with dummy matmuls
            # (no DMA dependency), then project Q stripe 0, K group 0 and
            # V tiles 0-5 as their inputs land.
            with tc.tile_pool(name="psA", bufs=8,
                              space=bass.MemorySpace.PSUM) as psA:
                # ~8.5us of dependency-free matmuls: warms the HAM clock
                # gate AND covers the ~7us DMA/preamble startup so real
                # projections start the moment their data lands.
                warm = psA.tile([128, QW], F32, tag="psA", name="warm")
                for i in range(150):
                    nc.tensor.matmul(
                        warm[0:DK, 0:DK], ones_bf[:, 0:DK],
                        ones_bf[:, 0:DK], start=True, stop=True)
                for m in range(2):
                    ps_q = psA.tile([128, QW], F32, tag="psA", name=f"psq{m}")
                    for fc in range(NF):
                        q_project(0, m, ps_q, fc)
                for m in range(2):
                    ps_k = psA.tile([128, QW], F32, tag="psA",
                                    name=f"psk0_{m}")
                    for fc in range(NF):
                        k_project(0, m, ps_k, fc)
                for tb in range(6):
                    ps_v = psA.tile([128, QW], F32, tag="psA",
                                    name=f"psv{tb}")
                    for dc in range(NF):
                        v_project(tb, ps_v, dc)

            # ---- phase B: striped attention with PE fillers -------------
            with contextlib.ExitStack() as _stB:
                def _poolB(**kw):
                    return _stB.enter_context(tc.tile_pool(**kw))

                es_pool = _poolB(name="ep", bufs=5)
                ub_pool = _poolB(name="ubp", bufs=8)
                rs_pool = _poolB(name="rsp", bufs=2)
                ob_pool = _poolB(name="obp", bufs=4)
                psS = _poolB(name="psS", bufs=2,
                             space=bass.MemorySpace.PSUM)
                psO = _poolB(name="psO", bufs=2,
                             space=bass.MemorySpace.PSUM)
                psF = _poolB(name="psF", bufs=2,
                             space=bass.MemorySpace.PSUM)
                ub_tiles = {}     # (qs, hp, hh) -> [64, 512] f32 tile
                rs_tiles = {}     # qs -> [128, 512] f32 rowsum-spread tile

                fstate = {}

                def qproj_fillers(s):
                    fs = []
                    for m in range(2):
                        def mk(mm, fc):
                            def f():
                                if fc == 0:
                                    fstate['q', mm] = psF.tile(
                                        [128, QW], F32, tag="psF",
                                        name=f"psq{s}_{mm}")
                                q_project(s, mm, fstate['q', mm], fc)
                            return f
                        for fc in range(NF):
                            fs.append(mk(m, fc))
                    return fs

                def kq_fillers(qh, m):
                    """K projection of column-group qh, head-pair tile m
                    (2 chunk-pairs per filler)."""
                    def mk(fp):
                        def f():
                            if fp == 0:
                                fstate['k', qh, m] = psF.tile(
                                    [128, QW], F32, tag="psF",
                                    name=f"psk{qh}_{m}")
                            for fc in (2 * fp, 2 * fp + 1):
                                k_project(qh, m, fstate['k', qh, m], fc)
                        return f
                    return [mk(fp) for fp in range(4)]

                def vtb_fillers(tb):
                    """V projection of k-tile tb (4 chunk-pairs/filler)."""
                    def mk(dp):
                        def f():
                            if dp == 0:
                                fstate['v', tb] = psF.tile(
                                    [128, QW], F32, tag="psF",
                                    name=f"psv{tb}")
                            for dc in range(4 * dp, 4 * dp + 4):
                                v_project(tb, fstate['v', tb], dc)
                        return f
                    return [mk(0), mk(1)]

                def recip_fillers(s, hps=(0, 1)):
                    """Reciprocal + normalize for stripe s (rowsums at
                    partitions 32h of rs_tiles[s]).  ACT cost depends only
                    on per-lane depth (512), so Ln/Exp run directly on the
                    [32h, q] layout and the selector matmul consumes the
                    result as-is -- no transposes needed.  The caller must
                    space fs[0] a few pops after the rowsum drains."""
                    fs = []

                    def t2():
                        lnr = rs_pool.tile([128, QW], F32, tag="rsT",
                                           name=f"lnr{s}")
                        nc.scalar.activation(lnr[:], rs_tiles[s][:], AFT.Ln)
                        rinv = rs_pool.tile([128, QW], BF16, tag="rinv",
                                            name=f"rinv{s}")
                        nc.scalar.activation(rinv[:], lnr[:],
                                             AFT.Exp, scale=-1.0)
                        recip_fillers.rinvT = rinv
                    fs.append(t2)

                    def mk_bcast(hp):
                        def f():
                            r_bc = psF.tile([128, QW], F32, tag="psF",
                                            name=f"rbc{s}_{hp}")
                            nc.tensor.matmul(
                                r_bc[:],
                                sel[hp][:],
                                recip_fillers.rinvT[:],
                                start=True, stop=True)
                            for hh in range(2):
                                nc.vector.tensor_mul(
                                    OTs[s][hp][hh * DK:(hh + 1) * DK, :],
                                    ub_tiles.pop((s, hp, hh))[0:DK, :],
                                    r_bc[hh * DK:(hh + 1) * DK, :])
                        return f
                    for hp in hps:
                        fs.append(mk_bcast(hp))
                    return fs
                recip_fillers.rinvT = None

                def outproj_fillers(s):
                    """Each (tt, ei) unit is split into a matmul closure
                    and a drain closure so the PE filler bursts stay
                    fine-grained inside the exp-bound attention cadence."""
                    fs = []

                    def mk_mm(tt, ei):
                        def f():
                            if ei == 0:
                                outproj_fillers.ob = ob_pool.tile(
                                    [128, D], BF16, tag="ob",
                                    name=f"ob{s}_{tt}")
                            f_ps = psF.tile([128, QW], F32, tag="psF",
                                            name=f"fps{s}_{tt}_{ei}")
                            fstate['op'] = f_ps
                            for m in range(2):
                                nc.tensor.matmul(
                                    f_ps[:],
                                    OTs[s][m][:, tt * 128:(tt + 1) * 128],
                                    WO[m][:, ei * QW:(ei + 1) * QW],
                                    start=(m == 0), stop=(m == 1))
                        return f

                    def mk_drain(tt, ei):
                        def f():
                            ob = outproj_fillers.ob
                            f_ps = fstate.pop('op')
                            if s == NQS - 1 and (tt + ei) % 2 == 0:
                                # tail: alternate drains between the idle
                                # ACT queue and DVE so they run 2-wide
                                nc.scalar.activation(
                                    ob[:, ei * QW:(ei + 1) * QW], f_ps[:],
                                    AFT.Copy)
                            else:
                                nc.vector.tensor_copy(
                                    ob[:, ei * QW:(ei + 1) * QW], f_ps[:])
                            if ei == 1:
                                t0 = (s * 4 + tt) * 128
                                nc.sync.dma_start(out[t0:t0 + 128, :], ob[:])
                        return f
                    for tt in range(4):
                        for ei in range(2):
                            fs.append(mk_mm(tt, ei))
                            fs.append(mk_drain(tt, ei))
                    return fs
                outproj_fillers.ob = None

                # flat (qs, hp, kt) stream: aV is emitted 1-3 steps behind
                # scores/exp so the FIFO PE queue never waits on an exp
                # before issuing independent scores work.  At block starts
                # the hold-back deepens to 3 so the previous block's DVE
                # drains (which gate aV(kt0) via o_ps buffer reuse) finish
                # under the run-ahead scores instead of stalling the PE.
                fillers = deque()
                pending = deque()  # (qs, hp, o_ps, es, kt)

                def flush_one():
                    pqs, php, po_ps, pes, pkt = pending.popleft()
                    for hh in range(2):
                        h = php * 2 + hh
                        nc.tensor.matmul(
                            po_ps[hh][0:DK + 1, :],
                            VE[pkt][:, h * (DK + 1):(h + 1) * (DK + 1)],
                            pes[:, hh * QW:(hh + 1) * QW],
                            start=(pkt == 0), stop=(pkt == NKT - 1))
                    if pkt == NKT - 1:
                        # drain O^T + rowsum row; heads at partitions 32h.
                        # The very last block's drains go on the otherwise
                        # idle ACT queue to shorten the serial tail.
                        last = pqs == NQS - 1 and php == 1
                        for hh in range(2):
                            h = php * 2 + hh
                            u = ub_pool.tile([128, QW], F32, tag="ub",
                                             name=f"ub{pqs}_{php}_{hh}")
                            if last and hh == 1:
                                nc.scalar.activation(
                                    u[0:DK, :], po_ps[hh][0:DK, :],
                                    AFT.Copy)
                                nc.scalar.activation(
                                    rs_tiles[pqs][32 * h:32 * h + 1, :],
                                    po_ps[hh][DK:DK + 1, :], AFT.Copy)
                            else:
                                nc.vector.tensor_copy(
                                    u[0:DK, :], po_ps[hh][0:DK, :])
                                nc.vector.tensor_copy(
                                    rs_tiles[pqs][32 * h:32 * h + 1, :],
                                    po_ps[hh][DK:DK + 1, :])
                            ub_tiles[(pqs, php, hh)] = u
                        if pqs == NQS - 1 and php == 0:
                            # last stripe: overlap hp0's half of the
                            # reciprocal under hp1's attention
                            fillers.extend([spacer] * 3)
                            fillers.extend(recip_fillers(pqs, hps=(0,)))

                def spacer():
                    pass

                for qs in range(NQS):
                    rf = recip_fillers(qs - 1) if qs > 0 else []
                    qp = qproj_fillers(qs + 1) if qs < NQS - 1 else []
                    if qs == 0:
                        # remaining input projections ride along stripe 0
                        # (2 filler pops per kt), ordered so every tile's
                        # drain is emitted before its first consumer
                        fillers.extend(kq_fillers(1, 0))
                        for tb in range(6, 10):
                            fillers.extend(vtb_fillers(tb))
                        fillers.extend(kq_fillers(2, 0))
                        fillers.extend(kq_fillers(3, 0))
                        for tb in range(10, NKT):
                            fillers.extend(vtb_fillers(tb))
                        for qh in range(1, 4):
                            fillers.extend(kq_fillers(qh, 1))
                        fillers.extend(qp)
                    elif rf:
                        if qp:
                            fillers.extend(qp[0:8])  # qproj m0 (pins psF)
                        else:
                            fillers.extend([spacer] * 4)
                        fillers.extend(rf)          # Ln/Exp + bcasts
                        fillers.extend(outproj_fillers(qs - 1))
                        fillers.extend(qp[8:16])    # qproj m1
                    else:
                        fillers.extend(qp)

                    rs_t = rs_pool.tile([128, QW], F32, tag="rs",
                                        name=f"rs{qs}")
                    nc.gpsimd.memset(rs_t[:], 1.0)
                    rs_tiles[qs] = rs_t

                    for hp in range(2):
                        o_ps = [psO.tile([128, QW], F32, tag="psO",
                                         name=f"o{qs}_{hp}_{i}")
                                for i in range(2)]
                        for kt in range(NKT):
                            sc = psS.tile([128, 2 * QW], F32, tag="psS",
                                          name=f"s{qs}_{hp}_{kt}")
                            for hh in range(2):
                                lo = hh * DK
                                nc.tensor.matmul(
                                    sc[:, hh * QW:(hh + 1) * QW],
                                    KT[hp][lo:lo + DK,
                                           kt * 128:(kt + 1) * 128],
                                    QTs[qs][hp][lo:lo + DK, :],
                                    start=True, stop=True)
                            es = es_pool.tile([128, 2 * QW], BF16, tag="es",
                                              name=f"e{qs}_{hp}_{kt}")
                            nc.scalar.activation(es[:], sc[:], AFT.Exp,
                                                 scale=float(SCALE))
                            # flush older blocks now; hold up to 3 of the
                            # current block while kt < 3
                            while pending and pending[0][0:2] != (qs, hp):
                                flush_one()
                            pending.append((qs, hp, o_ps, es, kt))
                            target = 3 if kt < 3 else (2 if kt < 5 else 1)
                            while len(pending) > target:
                                flush_one()
                            # NOTE: draining leftover fillers faster near
                            # the stripe seam (2 pops over the last kts)
                            # was tried and regressed 211us -> 252us; the
                            # single-pop cadence is load-bearing.
                            for _ in range(2 if qs == 0 else 1):
                                if fillers:
                                    fillers.popleft()()
                    # leftover fillers must land before the next stripe's
                    # scores read tiles they write (QTs of qs+1)
                    while fillers:
                        fillers.popleft()()

                # tail: flush last aV + drains, hp1 dance, outproj.
                # ~4.3us of dependency-free matmuls span the PE-idle
                # reciprocal-dance window so the HAM clock gate stays at
                # 2.4 GHz for the final output-projection matmuls.
                while pending:
                    flush_one()
                warm2 = psF.tile([128, QW], F32, tag="psF", name="warm2")
                for i in range(20):
                    nc.tensor.matmul(
                        warm2[0:DK, :], ones_bf[:, 0:DK], KT[0][:, 0:QW],
                        start=True, stop=True)
                for f in recip_fillers(NQS - 1, hps=(1,)):
                    f()
                for f in outproj_fillers(NQS - 1):
                    f()

    from concourse.bacc import get_activation_tables
    import bass_rust as _br
    _combined = "natural_log_exp_and_others"
    _tabs = []
    for _name, _fns in get_activation_tables(nc.m.arch).items():
        if _name != _combined:
            _fns = _fns - {AFT.Exp, AFT.Ln}
        _tabs.append((_name, _fns))
    _br.insert_act_table_loads(nc, _tabs)
    nc.compile()
    return nc


def _numpy_reference(q, k, v, mask, Wq, bq, Wk, bk, Wv, bv, Wo, bo):
    """Fallback for a non-trivial mask (never hit with the stock inputs)."""
    Bn, Tn, _ = q.shape
    H, dk = HEADS, DK

    def split(x):
        return x.reshape(Bn, Tn, H, dk).transpose(0, 2, 1, 3)

    qh = split(q @ Wq + bq)
    kh = split(k @ Wk + bk)
    vh = split(v @ Wv + bv)
    s = np.einsum("bhqd,bhkd->bhqk", qh, kh) / np.sqrt(np.float32(dk))
    s = np.where(mask, s, -np.inf)
    s = s - s.max(axis=-1, keepdims=True)
    e = np.exp(s)
    a = e / e.sum(axis=-1, keepdims=True)
    o = np.einsum("bhqk,bhkd->bhqd", a, vh)
    o = o.transpose(0, 2, 1, 3).reshape(Bn, Tn, H * dk)
    return (o @ Wo + bo).astype(np.float32)


def kernel(q, k, v, mask, Wq, bq, Wk, bk, Wv, bv, Wo, bo):
    global LAST_RESULTS
    q = np.asarray(q, np.float32)
    k = np.asarray(k, np.float32)
    v = np.asarray(v, np.float32)
    mask = np.asarray(mask, bool)
    Wq, bq = np.asarray(Wq, np.float32), np.asarray(bq, np.float32)
    Wk, bk = np.asarray(Wk, np.float32), np.asarray(bk, np.float32)
    Wv, bv = np.asarray(Wv, np.float32), np.asarray(bv, np.float32)
    Wo, bo = np.asarray(Wo, np.float32), np.asarray(bo, np.float32)

    if not mask.all():
        return _numpy_reference(q, k, v, mask, Wq, bq, Wk, bk, Wv, bv, Wo, bo)

    nc = _build_program()

    # host-side sharding; activations packed chunk-major per column
    # group (see the dram parameter comments in _build_program)
    def pack_cols(xT_b, w):
        ng = T // w
        return np.ascontiguousarray(
            xT_b.reshape(NF, 128, ng, w).transpose(2, 1, 0, 3)
            .reshape(ng, 128, NF * w))

    xP = {}
    for b in range(B):
        xq_t, xk_t, xv_t = (x[b].T.astype(BF) for x in (q, k, v))
        xP[b] = (pack_cols(xq_t, QW), pack_cols(xk_t, QW),
                 pack_cols(xv_t, 128))

    def w_chunks(W, g):
        # (1024, 256) head-group slice -> [128, 8*256] chunk-major layout
        Wg = W[:, g * GD:(g + 1) * GD]
        return np.ascontiguousarray(
            Wg.reshape(NF, 128, GD).transpose(1, 0, 2)
            .reshape(128, NF * GD).astype(BF))

    in_maps = []
    for c in range(NCORES):
        b, g = divmod(c, GH)
        xq_t, xk_t, xv_t = xP[b]
        in_maps.append({
            "xq": xq_t, "xk": xk_t, "xv": xv_t,
            "wq": w_chunks(Wq, g), "wk": w_chunks(Wk, g),
            "wv": w_chunks(Wv, g),
            "wo": np.ascontiguousarray(
                Wo[g * GD:(g + 1) * GD, :].astype(BF)).reshape(2, 128, D),
            "bqv": np.ascontiguousarray(
                bq[g * GD:(g + 1) * GD].reshape(2, 128).T),
        })

    LAST_RESULTS = run_bass_kernel_spmd(
        nc, in_maps, list(range(NCORES)),
        trace=bool(os.environ.get("KERNEL_TRACE")))
    res = LAST_RESULTS.results

    const_row = (bv @ Wo + bo).astype(np.float32)  # attn rows sum to 1
    full = np.empty((B, T, D), np.float32)
    for b in range(B):
        acc = res[b * GH]["out"].astype(np.float32)
        for g in range(1, GH):
            acc = acc + res[b * GH + g]["out"].astype(np.float32)
        full[b] = acc + const_row
    return full


# revision 72
# speedup vs baseline: 1.2094x; 1.0066x over previous
"""Trainium2 Bass kernel: 16-head MHA (B=2, T=2048, D=1024, d_k=64).

Sharding (8 NeuronCores): data-parallel over the batch (2) x tensor-parallel
over head groups (4 groups of 4 heads).  Core c handles batch b = c//4 and
heads [4g, 4g+4) with g = c%4.  Each core computes its partial output
    sum_{h in group} softmax((q Wq_h + bq_h)(k Wk_h)^T / 8) (v Wv_h) Wo_h
and the host sums the 4 partials per batch and adds the constant row
bo + bv @ Wo once.  bk is dropped: with the all-ones mask it shifts every
score row by a per-row constant, which softmax ignores exactly.

Design notes (420us baseline -> ~211us):
  * every matmul operand is bf16 (FWL weight loads, fp32 PSUM
    accumulate); output DMA'd as bf16 and upconverted host-side.
    The two heads of a pair sit on partition halves 0:64 / 64:128, so
    their C=64 scores matmuls land on disjoint PE row-tiles (T0/T8) and
    execute CONCURRENTLY (measured: starts 3 ns apart) -- scores cost
    half the naive streaming time.  (Column-tiled pair splits of the
    C=128 projections were tried and serialize; only row tiles overlap.)
  * V is projected directly in [t, v-col] layout (stationary = x^T
    chunk, moving = Wv) -- no PE transposes.  Activations arrive via
    host-packed chunk-major layouts (xq/xk per 512-col group, xv per
    128-row k-tile) so each projection unit depends on ~1 MB of DMA,
    not the whole tensor.
  * attention runs in 512-wide q stripes; per (stripe, head-pair, kt):
    2 concurrent scores MMs -> one [128,1024] fp32 PSUM tile, one ACT
    exp -> bf16 es, 2 aV MMs accumulating into per-head [65,512] PSUM
    (the 65th V_ext ones-column yields softmax rowsums for free, which
    is column-optimal: a separate rowsum matmul would stream es again).
    PSUM: scores dbuf 2x2 banks + O-accum 2 + filler 2 = 8 exactly.
  * one flat software-pipelined (stripe, hp, kt) stream: aV runs 1-3
    steps behind scores/exp (deepened at block starts) so the FIFO PE
    queue never waits on an exp or on the previous block's drains.
  * the PE is kept dense -- and the HAM clock gate at 2.4 GHz -- by
    emitting everything else as fillers inside the ACT-bound attention
    loop: K groups 1-3 + V tiles 6-15 + Q stripe s+1 projections, the
    reciprocal dance, and stripe s-1's output projection, each unit
    ordered so its drain is emitted before its first consumer.
    ~150 dependency-free warm-up matmuls cover the ~7us DMA startup.
  * softmax denominators: rowsum rows are collected at partitions 32h,
    DVE 32-block transposed, one batched ACT Ln + Exp(-1) computes all
    reciprocals partition-parallel, transposed back, and broadcast
    across partitions by one C=128 selector matmul per head pair.
    The last stripe runs hp0's dance under hp1's attention and drains
    on the otherwise-idle ACT queue to shorten the serial tail.
"""

import functools
import os
from collections import deque

import ml_dtypes
import numpy as np

import concourse.bass as bass
import concourse.mybir as mybir
import concourse.tile as tile
from concourse import bacc
from concourse.bass_utils import run_bass_kernel_spmd

F32 = mybir.dt.float32
F32R = mybir.dt.float32r
BF16 = mybir.dt.bfloat16
AFT = mybir.ActivationFunctionType
BF = ml_dtypes.bfloat16

D = 1024          # model dim
T = 2048          # sequence length
B = 2             # batch
HEADS = 16        # total heads
DK = 64           # head dim
NCORES = 8
GH = 4            # heads per core
GD = GH * DK      # 256 projection cols per core
NF = D // 128     # 8 contraction chunks
NKT = T // 128    # 16 k tiles
NQS = 4           # 512-wide q stripes
QW = T // NQS     # 512
SCALE = 1.0 / np.sqrt(np.float32(DK))  # 1/8

# Results of the last run (for test harness introspection: exec_time_ns etc.)
LAST_RESULTS = None


@functools.lru_cache(maxsize=1)
def _build_program():
    nc = bacc.Bacc("TRN2", target_bir_lowering=False, debug=False,
                   num_devices=NCORES)

    # host-packed activation layouts (see _pack_* in kernel()):
    #   xq[s]  = [128, NF*QW]  q-stripe s, chunk-major (8 KiB DMA lines)
    #   xk[qh] = [128, NF*QW]  k column-group qh, chunk-major
    #   xv[tb] = [128, NF*128] k-tile tb, chunk-major (2 KiB lines)
    xq = nc.declare_dram_parameter("xq", [NQS, 128, NF * QW], BF16,
                                   isOutput=False)
    xk = nc.declare_dram_parameter("xk", [4, 128, NF * QW], BF16,
                                   isOutput=False)
    xv = nc.declare_dram_parameter("xv", [NKT, 128, NF * 128], BF16,
                                   isOutput=False)
    wq = nc.declare_dram_parameter("wq", [128, NF * GD], BF16, isOutput=False)
    wk = nc.declare_dram_parameter("wk", [128, NF * GD], BF16, isOutput=False)
    wv = nc.declare_dram_parameter("wv", [128, NF * GD], BF16, isOutput=False)
    wo = nc.declare_dram_parameter("wo", [2, 128, D], BF16, isOutput=False)
    bqv = nc.declare_dram_parameter("bqv", [128, 2], F32, isOutput=False)
    out = nc.declare_dram_parameter("out", [T, D], BF16, isOutput=True)

    import contextlib
    with tile.TileContext(nc) as tc, contextlib.ExitStack() as _st:
        # ---- persistent pools -------------------------------------------
        def _pool(**kw):
            return _st.enter_context(tc.tile_pool(**kw))

        if True:
            kt_pool = _pool(name="kt", bufs=2)
            vext_pool = _pool(name="vext", bufs=NKT)
            qts_pool = _pool(name="qts", bufs=NQS * 2)
            ots_pool = _pool(name="ots", bufs=NQS * 2)
            w_pool = _pool(name="wts", bufs=3)
            wo_pool = _pool(name="wop", bufs=2)
            xq_pool = _pool(name="xq", bufs=NQS)
            xk_pool = _pool(name="xk", bufs=4)
            xv_pool = _pool(name="xv", bufs=NKT)
            const_pool = _pool(name="const", bufs=1)
            ones_f32 = const_pool.tile([128, GH], F32, tag="ones32")
            nc.gpsimd.memset(ones_f32[:], 1.0)
            ones_bf = const_pool.tile([128, DK], BF16, tag="onesbf")
            nc.gpsimd.memset(ones_bf[:], 1.0)
            # head-pair selectors: sel[hp][c, m] = (c == 32*(hp*2 + m//64)),
            # i.e. r_bc[m, :] = rinvT[32*head(m), :] after the C=128 matmul
            sel = [const_pool.tile([128, 128], BF16, tag=f"sel{hp}",
                                   name=f"sel{hp}") for hp in range(2)]
            for hp in range(2):
                nc.gpsimd.memset(sel[hp][:], 0.0)
                for hh in range(2):
                    c = 32 * (hp * 2 + hh)
                    nc.vector.tensor_copy(
                        sel[hp][c:c + 1, hh * DK:(hh + 1) * DK],
                        ones_bf[0:1, 0:DK])
            bqv_sb = const_pool.tile([128, 2], F32, tag="bqv")
            nc.sync.dma_start(bqv_sb[:], bqv[:])

            KT = [kt_pool.tile([128, T], BF16, tag="kt", name=f"kt{m}")
                  for m in range(2)]
            VE = [vext_pool.tile([128, GH * (DK + 1)], BF16, tag="vext",
                                 name=f"ve{i}") for i in range(NKT)]
            # per-stripe Q^T and O^T tiles (heads of pair hp stacked 64+64)
            QTs = [[qts_pool.tile([128, QW], BF16, tag="qts",
                                  name=f"qt{s}_{m}") for m in range(2)]
                   for s in range(NQS)]
            OTs = [[ots_pool.tile([128, QW], BF16, tag="ots",
                                  name=f"ot{s}_{m}") for m in range(2)]
                   for s in range(NQS)]
            WO = [wo_pool.tile([128, D], BF16, tag="wop", name=f"wo{m}")
                  for m in range(2)]

            wq_sb = w_pool.tile([128, NF * GD], BF16, tag="w", name="wq_sb")
            wk_sb = w_pool.tile([128, NF * GD], BF16, tag="w", name="wk_sb")
            wv_sb = w_pool.tile([128, NF * GD], BF16, tag="w", name="wv_sb")
            XQs = [xq_pool.tile([128, NF * QW], BF16, tag="xq",
                                name=f"xqs{s}") for s in range(NQS)]
            XKq = [xk_pool.tile([128, NF * QW], BF16, tag="xk",
                                name=f"xkq{i}") for i in range(4)]
            XVt = [xv_pool.tile([128, NF * 128], BF16, tag="xv",
                                name=f"xvt{i}") for i in range(NKT)]

            # V_ext ones columns (persistent; written once, no DMA dep)
            for tb in range(NKT):
                ve_r = VE[tb][:].rearrange("p (h x) -> p h x", x=DK + 1)
                nc.vector.tensor_copy(
                    ve_r[:, :, DK:DK + 1],
                    ones_f32[:].rearrange("p (h x) -> p h x", x=1))

            # ---- DMA in, ordered to match the projection filler schedule
            # (upfront: Q stripe 0, K group 0, V tiles 0-5; the rest lands
            # under stripe 0's attention).  All transfers stay on the sync
            # engine's HWDGE queue: routing part of them through the
            # scalar engine's queue was tried and regressed ~10us (the
            # descriptor issue traffic perturbs the exp stream).
            nc.sync.dma_start(wq_sb[:], wq[:])
            nc.sync.dma_start(XQs[0][:], xq[0])
            nc.sync.dma_start(wk_sb[:], wk[:])
            nc.sync.dma_start(XKq[0][:], xk[0])
            nc.sync.dma_start(wv_sb[:], wv[:])
            for tb in range(6):
                nc.sync.dma_start(XVt[tb][:], xv[tb])
            nc.sync.dma_start(XKq[1][:], xk[1])
            for tb in range(6, 10):
                nc.sync.dma_start(XVt[tb][:], xv[tb])
            nc.sync.dma_start(XKq[2][:], xk[2])
            nc.sync.dma_start(XKq[3][:], xk[3])
            for tb in range(10, NKT):
                nc.sync.dma_start(XVt[tb][:], xv[tb])
            for s in range(1, NQS):
                nc.sync.dma_start(XQs[s][:], xq[s])
            nc.sync.dma_start(WO[0][:], wo[0])
            nc.sync.dma_start(WO[1][:], wo[1])

            # (A same-bank row-tiled pair variant of these projections --
            # start=False accumulation onto a pre-zeroed bank -- was
            # tried and hangs the device at runtime; keep full-C MMs.)
            def q_project(s, m, ps_q, fc):
                nc.tensor.matmul(
                    ps_q[:],
                    wq_sb[:, fc * GD + m * 128:fc * GD + (m + 1) * 128],
                    XQs[s][:, fc * QW:(fc + 1) * QW],
                    start=(fc == 0), stop=(fc == NF - 1))
                if fc == NF - 1:
                    nc.vector.tensor_scalar_add(
                        QTs[s][m][:], ps_q[:], bqv_sb[:, m:m + 1])

            def k_project(qh, m, ps_k, fc):
                nc.tensor.matmul(
                    ps_k[:],
                    wk_sb[:, fc * GD + m * 128:fc * GD + (m + 1) * 128],
                    XKq[qh][:, fc * QW:(fc + 1) * QW],
                    start=(fc == 0), stop=(fc == NF - 1))
                if fc == NF - 1:
                    nc.vector.tensor_copy(
                        KT[m][:, qh * QW:(qh + 1) * QW], ps_k[:])

            def v_project(tb, ps_v, dc):
                nc.tensor.matmul(
                    ps_v[:, 0:GD],
                    XVt[tb][:, dc * 128:(dc + 1) * 128],
                    wv_sb[:, dc * GD:(dc + 1) * GD],
                    start=(dc == 0), stop=(dc == NF - 1))
                if dc == NF - 1:
                    ve_r = VE[tb][:].rearrange("p (h x) -> p h x", x=DK + 1)
                    nc.vector.tensor_copy(
                        ve_r[:, :, 0:DK],
                        ps_v[:, 0:GD].rearrange("p (h x) -> p h x", x=DK))

            # ---- phase A: warm the HAM clock gate with dummy matmuls
            # (no DMA dependency), then project Q stripe 0, K group 0 and
            # V tiles 0-5 as their inputs land.
            with tc.tile_pool(name="psA", bufs=8,
                              space=bass.MemorySpace.PSUM) as psA:
                # ~8.5us of dependency-free matmuls: warms the HAM clock
                # gate AND covers the ~7us DMA/preamble startup so real
                # projections start the moment their data lands.
                warm = psA.tile([128, QW], F32, tag="psA", name="warm")
                for i in range(150):
                    nc.tensor.matmul(
                        warm[0:DK, 0:DK], ones_bf[:, 0:DK],
                        ones_bf[:, 0:DK], start=True, stop=True)
                for m in range(2):
                    ps_q = psA.tile([128, QW], F32, tag="psA", name=f"psq{m}")
                    for fc in range(NF):
                        q_project(0, m, ps_q, fc)
                for m in range(2):
                    ps_k = psA.tile([128, QW], F32, tag="psA",
                                    name=f"psk0_{m}")
                    for fc in range(NF):
                        k_project(0, m, ps_k, fc)
                for tb in range(6):
                    ps_v = psA.tile([128, QW], F32, tag="psA",
                                    name=f"psv{tb}")
                    for dc in range(NF):
                        v_project(tb, ps_v, dc)

            # ---- phase B: striped attention with PE fillers -------------
            with contextlib.ExitStack() as _stB:
                def _poolB(**kw):
                    return _stB.enter_context(tc.tile_pool(**kw))

                es_pool = _poolB(name="ep", bufs=5)
                ub_pool = _poolB(name="ubp", bufs=8)
                rs_pool = _poolB(name="rsp", bufs=2)
                ob_pool = _poolB(name="obp", bufs=4)
                psS = _poolB(name="psS", bufs=2,
                             space=bass.MemorySpace.PSUM)
                psO = _poolB(name="psO", bufs=2,
                             space=bass.MemorySpace.PSUM)
                psF = _poolB(name="psF", bufs=2,
                             space=bass.MemorySpace.PSUM)
                ub_tiles = {}     # (qs, hp, hh) -> [64, 512] f32 tile
                rs_tiles = {}     # qs -> [128, 512] f32 rowsum-spread tile

                fstate = {}

                def qproj_fillers(s):
                    fs = []
                    for m in range(2):
                        def mk(mm, fc):
                            def f():
                                if fc == 0:
                                    fstate['q', mm] = psF.tile(
                                        [128, QW], F32, tag="psF",
                                        name=f"psq{s}_{mm}")
                                q_project(s, mm, fstate['q', mm], fc)
                            return f
                        for fc in range(NF):
                            fs.append(mk(m, fc))
                    return fs

                def kq_fillers(qh, m):
                    """K projection of column-group qh, head-pair tile m
                    (2 chunk-pairs per filler)."""
                    def mk(fp):
                        def f():
                            if fp == 0:
                                fstate['k', qh, m] = psF.tile(
                                    [128, QW], F32, tag="psF",
                                    name=f"psk{qh}_{m}")
                            for fc in (2 * fp, 2 * fp + 1):
                                k_project(qh, m, fstate['k', qh, m], fc)
                        return f
                    return [mk(fp) for fp in range(4)]

                def vtb_fillers(tb):
                    """V projection of k-tile tb (4 chunk-pairs/filler)."""
                    def mk(dp):
                        def f():
                            if dp == 0:
                                fstate['v', tb] = psF.tile(
                                    [128, QW], F32, tag="psF",
                                    name=f"psv{tb}")
                            for dc in range(4 * dp, 4 * dp + 4):
                                v_project(tb, fstate['v', tb], dc)
                        return f
                    return [mk(0), mk(1)]

                def recip_fillers(s, hps=(0, 1)):
                    """Reciprocal + normalize for stripe s (rowsums at
                    partitions 32h of rs_tiles[s]).  ACT cost depends only
                    on per-lane depth (512), so Ln/Exp run directly on the
                    [32h, q] layout and the selector matmul consumes the
                    result as-is -- no transposes needed.  The caller must
                    space fs[0] a few pops after the rowsum drains."""
                    fs = []

                    def t2():
                        lnr = rs_pool.tile([128, QW], F32, tag="rsT",
                                           name=f"lnr{s}")
                        nc.scalar.activation(lnr[:], rs_tiles[s][:], AFT.Ln)
                        rinv = rs_pool.tile([128, QW], BF16, tag="rinv",
                                            name=f"rinv{s}")
                        nc.scalar.activation(rinv[:], lnr[:],
                                             AFT.Exp, scale=-1.0)
                        recip_fillers.rinvT = rinv
                    fs.append(t2)

                    def mk_bcast(hp):
                        def f():
                            r_bc = psF.tile([128, QW], F32, tag="psF",
                                            name=f"rbc{s}_{hp}")
                            nc.tensor.matmul(
                                r_bc[:],
                                sel[hp][:],
                                recip_fillers.rinvT[:],
                                start=True, stop=True)
                            for hh in range(2):
                                nc.vector.tensor_mul(
                                    OTs[s][hp][hh * DK:(hh + 1) * DK, :],
                                    ub_tiles.pop((s, hp, hh))[0:DK, :],
                                    r_bc[hh * DK:(hh + 1) * DK, :])
                        return f
                    for hp in hps:
                        fs.append(mk_bcast(hp))
                    return fs
                recip_fillers.rinvT = None

                def outproj_fillers(s):
                    """Each (tt, ei) unit is split into a matmul closure
                    and a drain closure so the PE filler bursts stay
                    fine-grained inside the exp-bound attention cadence."""
                    fs = []

                    def mk_mm(tt, ei):
                        def f():
                            if ei == 0:
                                outproj_fillers.ob = ob_pool.tile(
                                    [128, D], BF16, tag="ob",
                                    name=f"ob{s}_{tt}")
                            f_ps = psF.tile([128, QW], F32, tag="psF",
                                            name=f"fps{s}_{tt}_{ei}")
                            fstate['op'] = f_ps
                            for m in range(2):
                                nc.tensor.matmul(
                                    f_ps[:],
                                    OTs[s][m][:, tt * 128:(tt + 1) * 128],
                                    WO[m][:, ei * QW:(ei + 1) * QW],
                                    start=(m == 0), stop=(m == 1))
                        return f

                    def mk_drain(tt, ei):
                        def f():
                            ob = outproj_fillers.ob
                            f_ps = fstate.pop('op')
                            if s == NQS - 1 and (tt + ei) % 2 == 0:
                                # tail: alternate drains between the idle
                                # ACT queue and DVE so they run 2-wide
                                nc.scalar.activation(
                                    ob[:, ei * QW:(ei + 1) * QW], f_ps[:],
                                    AFT.Copy)
                            else:
                                nc.vector.tensor_copy(
                                    ob[:, ei * QW:(ei + 1) * QW], f_ps[:])
                            if ei == 1:
                                t0 = (s * 4 + tt) * 128
                                nc.sync.dma_start(out[t0:t0 + 128, :], ob[:])
                        return f
                    for tt in range(4):
                        for ei in range(2):
                            fs.append(mk_mm(tt, ei))
                            fs.append(mk_drain(tt, ei))
                    return fs
                outproj_fillers.ob = None

                # flat (qs, hp, kt) stream: aV is emitted 1-3 steps behind
                # scores/exp so the FIFO PE queue never waits on an exp
                # before issuing independent scores work.  At block starts
                # the hold-back deepens to 3 so the previous block's DVE
                # drains (which gate aV(kt0) via o_ps buffer reuse) finish
                # under the run-ahead scores instead of stalling the PE.
                fillers = deque()
                pending = deque()  # (qs, hp, o_ps, es, kt)

                def flush_one():
                    pqs, php, po_ps, pes, pkt = pending.popleft()
                    for hh in range(2):
                        h = php * 2 + hh
                        nc.tensor.matmul(
                            po_ps[hh][0:DK + 1, :],
                            VE[pkt][:, h * (DK + 1):(h + 1) * (DK + 1)],
                            pes[:, hh * QW:(hh + 1) * QW],
                            start=(pkt == 0), stop=(pkt == NKT - 1))
                    if pkt == NKT - 1:
                        # drain O^T + rowsum row; heads at partitions 32h.
                        # The very last block's drains go on the otherwise
                        # idle ACT queue to shorten the serial tail.
                        last = pqs == NQS - 1 and php == 1
                        for hh in range(2):
                            h = php * 2 + hh
                            u = ub_pool.tile([128, QW], F32, tag="ub",
                                             name=f"ub{pqs}_{php}_{hh}")
                            if last and hh == 1:
                                nc.scalar.activation(
                                    u[0:DK, :], po_ps[hh][0:DK, :],
                                    AFT.Copy)
                                nc.scalar.activation(
                                    rs_tiles[pqs][32 * h:32 * h + 1, :],
                                    po_ps[hh][DK:DK + 1, :], AFT.Copy)
                            else:
                                nc.vector.tensor_copy(
                                    u[0:DK, :], po_ps[hh][0:DK, :])
                                nc.vector.tensor_copy(
                                    rs_tiles[pqs][32 * h:32 * h + 1, :],
                                    po_ps[hh][DK:DK + 1, :])
                            ub_tiles[(pqs, php, hh)] = u
                        if pqs == NQS - 1 and php == 0:
                            # last stripe: overlap hp0's half of the
                            # reciprocal under hp1's attention
                            fillers.extend([spacer] * 3)
                            fillers.extend(recip_fillers(pqs, hps=(0,)))

                def spacer():
                    pass

                for qs in range(NQS):
                    rf = recip_fillers(qs - 1) if qs > 0 else []
                    qp = qproj_fillers(qs + 1) if qs < NQS - 1 else []
                    if qs == 0:
                        # remaining input projections ride along stripe 0
                        # (2 filler pops per kt), ordered so every tile's
                        # drain is emitted before its first consumer
                        fillers.extend(kq_fillers(1, 0))
                        for tb in range(6, 10):
                            fillers.extend(vtb_fillers(tb))
                        fillers.extend(kq_fillers(2, 0))
                        fillers.extend(kq_fillers(3, 0))
                        for tb in range(10, NKT):
                            fillers.extend(vtb_fillers(tb))
                        for qh in range(1, 4):
                            fillers.extend(kq_fillers(qh, 1))
                        fillers.extend(qp)
                    elif rf:
                        if qp:
                            fillers.extend(qp[0:8])  # qproj m0 (pins psF)
                        else:
                            fillers.extend([spacer] * 4)
                        fillers.extend(rf)          # Ln/Exp + bcasts
                        fillers.extend(outproj_fillers(qs - 1))
                        fillers.extend(qp[8:16])    # qproj m1
                    else:
                        fillers.extend(qp)

                    rs_t = rs_pool.tile([128, QW], F32, tag="rs",
                                        name=f"rs{qs}")
                    nc.gpsimd.memset(rs_t[:], 1.0)
                    rs_tiles[qs] = rs_t

                    for hp in range(2):
                        o_ps = [psO.tile([128, QW], F32, tag="psO",
                                         name=f"o{qs}_{hp}_{i}")
                                for i in range(2)]
                        for kt in range(NKT):
                            sc = psS.tile([128, 2 * QW], F32, tag="psS",
                                          name=f"s{qs}_{hp}_{kt}")
                            for hh in range(2):
                                lo = hh * DK
                                nc.tensor.matmul(
                                    sc[:, hh * QW:(hh + 1) * QW],
                                    KT[hp][lo:lo + DK,
                                           kt * 128:(kt + 1) * 128],
                                    QTs[qs][hp][lo:lo + DK, :],
                                    start=True, stop=True)
                            es = es_pool.tile([128, 2 * QW], BF16, tag="es",
                                              name=f"e{qs}_{hp}_{kt}")
                            nc.scalar.activation(es[:], sc[:], AFT.Exp,
                                                 scale=float(SCALE))
                            # flush older blocks now; hold up to 3 of the
                            # current block while kt < 3
                            while pending and pending[0][0:2] != (qs, hp):
                                flush_one()
                            pending.append((qs, hp, o_ps, es, kt))
                            target = 3 if kt < 3 else (2 if kt < 5 else 1)
                            while len(pending) > target:
                                flush_one()
                            # double-pop near the stripe seam so leftover
                            # fillers don't flush serially between the
                            # last aV and the next stripe's first scores
                            npop = 2 if (qs == 0 or
                                         (hp == 1 and kt >= NKT - 4)) else 1
                            for _ in range(npop):
                                if fillers:
                                    fillers.popleft()()
                    # leftover fillers must land before the next stripe's
                    # scores read tiles they write (QTs of qs+1)
                    while fillers:
                        fillers.popleft()()

                # tail: flush last aV + drains, hp1 dance, outproj.
                # ~4.3us of dependency-free matmuls span the PE-idle
                # reciprocal-dance window so the HAM clock gate stays at
                # 2.4 GHz for the final output-projection matmuls.
                while pending:
                    flush_one()
                warm2 = psF.tile([128, QW], F32, tag="psF", name="warm2")
                for i in range(20):
                    nc.tensor.matmul(
                        warm2[0:DK, :], ones_bf[:, 0:DK], KT[0][:, 0:QW],
                        start=True, stop=True)
                for f in recip_fillers(NQS - 1, hps=(1,)):
                    f()
                for f in outproj_fillers(NQS - 1):
                    f()

    from concourse.bacc import get_activation_tables
    import bass_rust as _br
    _combined = "natural_log_exp_and_others"
    _tabs = []
    for _name, _fns in get_activation_tables(nc.m.arch).items():
        if _name != _combined:
            _fns = _fns - {AFT.Exp, AFT.Ln}
        _tabs.append((_name, _fns))
    _br.insert_act_table_loads(nc, _tabs)
    nc.compile()
    return nc


def _numpy_reference(q, k, v, mask, Wq, bq, Wk, bk, Wv, bv, Wo, bo):
    """Fallback for a non-trivial mask (never hit with the stock inputs)."""
    Bn, Tn, _ = q.shape
    H, dk = HEADS, DK

    def split(x):
        return x.reshape(Bn, Tn, H, dk).transpose(0, 2, 1, 3)

    qh = split(q @ Wq + bq)
    kh = split(k @ Wk + bk)
    vh = split(v @ Wv + bv)
    s = np.einsum("bhqd,bhkd->bhqk", qh, kh) / np.sqrt(np.float32(dk))
    s = np.where(mask, s, -np.inf)
    s = s - s.max(axis=-1, keepdims=True)
    e = np.exp(s)
    a = e / e.sum(axis=-1, keepdims=True)
    o = np.einsum("bhqk,bhkd->bhqd", a, vh)
    o = o.transpose(0, 2, 1, 3).reshape(Bn, Tn, H * dk)
    return (o @ Wo + bo).astype(np.float32)


def kernel(q, k, v, mask, Wq, bq, Wk, bk, Wv, bv, Wo, bo):
    global LAST_RESULTS
    q = np.asarray(q, np.float32)
    k = np.asarray(k, np.float32)
    v = np.asarray(v, np.float32)
    mask = np.asarray(mask, bool)
    Wq, bq = np.asarray(Wq, np.float32), np.asarray(bq, np.float32)
    Wk, bk = np.asarray(Wk, np.float32), np.asarray(bk, np.float32)
    Wv, bv = np.asarray(Wv, np.float32), np.asarray(bv, np.float32)
    Wo, bo = np.asarray(Wo, np.float32), np.asarray(bo, np.float32)

    if not mask.all():
        return _numpy_reference(q, k, v, mask, Wq, bq, Wk, bk, Wv, bv, Wo, bo)

    nc = _build_program()

    # host-side sharding; activations packed chunk-major per column
    # group (see the dram parameter comments in _build_program)
    def pack_cols(xT_b, w):
        ng = T // w
        return np.ascontiguousarray(
            xT_b.reshape(NF, 128, ng, w).transpose(2, 1, 0, 3)
            .reshape(ng, 128, NF * w))

    xP = {}
    for b in range(B):
        xq_t, xk_t, xv_t = (x[b].T.astype(BF) for x in (q, k, v))
        xP[b] = (pack_cols(xq_t, QW), pack_cols(xk_t, QW),
                 pack_cols(xv_t, 128))

    def w_chunks(W, g):
        # (1024, 256) head-group slice -> [128, 8*256] chunk-major layout
        Wg = W[:, g * GD:(g + 1) * GD]
        return np.ascontiguousarray(
            Wg.reshape(NF, 128, GD).transpose(1, 0, 2)
            .reshape(128, NF * GD).astype(BF))

    in_maps = []
    for c in range(NCORES):
        b, g = divmod(c, GH)
        xq_t, xk_t, xv_t = xP[b]
        in_maps.append({
            "xq": xq_t, "xk": xk_t, "xv": xv_t,
            "wq": w_chunks(Wq, g), "wk": w_chunks(Wk, g),
            "wv": w_chunks(Wv, g),
            "wo": np.ascontiguousarray(
                Wo[g * GD:(g + 1) * GD, :].astype(BF)).reshape(2, 128, D),
            "bqv": np.ascontiguousarray(
                bq[g * GD:(g + 1) * GD].reshape(2, 128).T),
        })

    LAST_RESULTS = run_bass_kernel_spmd(
        nc, in_maps, list(range(NCORES)),
        trace=bool(os.environ.get("KERNEL_TRACE")))
    res = LAST_RESULTS.results

    const_row = (bv @ Wo + bo).astype(np.float32)  # attn rows sum to 1
    full = np.empty((B, T, D), np.float32)
    for b in range(B):
        acc = res[b * GH]["out"].astype(np.float32)
        for g in range(1, GH):
            acc = acc + res[b * GH + g]["out"].astype(np.float32)
        full[b] = acc + const_row
    return full
